# revision 1
# baseline (speedup 1.0000x reference)
"""AffinityCosineLoss on 8 Trainium2 NeuronCores — fp8 streaming matmul.

Math: with zn = l2norm(y_pred[:, :192]), latent = (zn@zn.T + 1)/2,
target[i,j] = 0.2 (both bg) / 0.01 (one bg) / lookup[y_i,y_j] (both valid),
loss = sum_{i<j} |latent - target| / (B*(B-1)/2).

The entire pairwise map latent - target is a single K=323 contraction
P.T @ Q, fully packed on the HOST (fp32 math, then fp8 cast):
  rows   0:192  P = zn_i.T            Q = 0.5 * zn_j.T
  row    192    P = 1                 Q = 0.5            (the +1/2 of latent)
  row    193    P = b_i               Q = -0.01 - 0.18*b_j
  row    194    P = 1                 Q = -0.01*b_j      (b = is_background)
  rows 195:323  P = onehot(y_i)       Q = -lookup[:, y_j] * valid_j
The asymmetric 1.0 x 0.5 const split keeps the fp8 constants exact.
K chunks: A = rows 0:128, B = rows 128:256, C = rows 256:323 (67).
Plain matmuls (no DoubleRow): full 128-col stationaries trigger the
compiler's Fast Weight Load, and --enable-ldw-opt dedupes the repeated
stationary across the slot-pair inner loop.

Sharding (triangle/cyclic): the 4096x4096 pair matrix is an 8x8 grid of
512x512 super-blocks. Core r computes blocks (r, (r+d) mod 8) for d=0..4;
the d=4 slot is zero-padded on cores 4..7. The x2 weight of off-diagonal
slots is baked into the Q columns (|2x| = 2|x|), so the device just
abs-sums everything. Host: total = sum - diag_correction, /2, /npairs.

Device: 20 out-tiles [128,512] in 3 waves of slots {0,1},{2,3},{4};
PSUM units [128,2,512] rotate through all 8 banks (tag bufs=4, first
rotation slot doubles as the PE-warmup target). Drains (abs + sum into
one acc column) alternate ACT (activation Abs accum_out) / DVE
(tensor_reduce) and overlap the next wave's matmuls.
"""

import functools

import ml_dtypes
import numpy as np

B = 4096
D = 256
L = 128
D_USE = 192  # int(D * 0.75)
NB = 8  # super-block grid (512 rows each)
BLK = B // NB  # 512
NSLOT = 5  # col slots per core (d = 0..4)
NCOL = NSLOT * BLK  # 2560
N_CORES = 8
NORM_EPS = 1e-8

KT = 323  # contraction rows
KC = 67  # rows of chunk C
NUNIT = 2 * NSLOT  # drain units: (slot, strip-half)

FP8 = ml_dtypes.float8_e4m3


def _build_bass():
    import concourse.bacc as bacc
    import concourse.mybir as mybir
    import concourse.tile as tile

    # NOTE: walrus --enable-ldw-opt rejects these fp8 InstLdweights
    # ("not compatible with LDW optimization"), so it stays off; the
    # PE's 64-deep reorder window still pulls LDWEIGHTS ahead.

    fp32 = mybir.dt.float32
    bf16 = mybir.dt.bfloat16
    f8 = mybir.dt.float8e4

    nc = bacc.Bacc("TRN2", debug=False, num_devices=N_CORES)

    # all 128-partition input in one tensor, all 67-partition input in the
    # other: 2 + 2 wave-sliced DMA issues move everything
    i128_d = nc.dram_tensor(
        "i128", [128, 1024 + NSLOT * 1024], f8, kind="ExternalInput"
    )
    i67_d = nc.dram_tensor("i67", [KC, BLK + NSLOT * BLK], f8, kind="ExternalInput")
    acc_d = nc.dram_tensor("acc", [128, NUNIT], fp32, kind="ExternalOutput")

    AX = mybir.AxisListType
    ALU = mybir.AluOpType
    ACTF = mybir.ActivationFunctionType

    with tile.TileContext(nc) as tc:
        with (
            tc.tile_pool(name="cst", bufs=1) as cst,
            tc.tile_pool(name="work", bufs=1) as work,
            tc.tile_pool(name="ps", bufs=1, space="PSUM") as pps,
        ):
            # ---- SBUF tiles: one tile per DMA so dependency tracking and
            # write-hazard windows are exact (a matmul only waits on — and
            # only aliases — its own wave's buffer) ----
            pabs = work.tile([128, 2, BLK], f8)
            pcs = work.tile([KC, BLK], f8)
            qabw = [
                work.tile([128, 2 if w < 2 else 1, 2, BLK], f8, name=f"qab{w}")
                for w in range(3)
            ]
            qcw = [
                work.tile([KC, 2 if w < 2 else 1, BLK], f8, name=f"qc{w}")
                for w in range(3)
            ]
            acc = work.tile([128, NUNIT], fp32)

            # ---- engine warmup + input DMAs ----
            # All input rides the fast sync HWDGE ring, need-ordered.
            # Scalar keeps only the warmup activation (Abs table) + drains.
            wz = cst.tile([128, 512], f8)
            nc.gpsimd.memset(wz[:], 0.0)
            wact = cst.tile([128, 1], fp32)
            nc.gpsimd.memset(wact[:], 1.0)

            def dma_ab(w):
                ab = slice(1024 + w * 2048, 1024 + min((w + 1) * 2048, 5120))
                nc.sync.dma_start(qabw[w][:], i128_d.ap()[:, ab])

            def dma_c(w):
                c = slice(BLK + w * 1024, BLK + min((w + 1) * 1024, NSLOT * BLK))
                nc.sync.dma_start(qcw[w][:], i67_d.ap()[:, c])

            nc.sync.dma_start(pabs[:], i128_d.ap()[:, 0:1024])
            nc.sync.dma_start(pcs[:], i67_d.ap()[:, 0:BLK])
            for w in range(3):
                dma_ab(w)
                dma_c(w)

            wabs = cst.tile([128, 1], fp32)
            nc.scalar.activation(wabs[:], wact[:], ACTF.Abs)

            wp = pps.tile([128, 2, BLK], fp32, tag="mm", bufs=4, name="wp")
            for wi in range(8):
                nc.tensor.matmul(
                    wp[:, wi % 2, :], wz[:, 0:128], wz[:], start=True, stop=True
                )

            # ---- main: 3 waves of slots {0,1}, {2,3}, {4} ----
            pending = []

            def drain(unit, u):
                if u % 2 == 1:
                    scr = work.tile([128, 2, BLK], bf16, tag="scr", bufs=2)
                    nc.scalar.activation(
                        scr[:], unit[:], ACTF.Abs, accum_out=acc[:, u : u + 1]
                    )
                else:
                    # DVE reduce is the faster drain; it takes the units on
                    # the bank-reuse critical edge (first unit of each wave)
                    nc.vector.tensor_reduce(
                        acc[:, u : u + 1],
                        unit[:],
                        axis=AX.XY,
                        op=ALU.add,
                        apply_absolute_value=True,
                    )

            for wave in ((0, 1), (2, 3), (4,)):
                units = {}
                for g in wave:
                    for h in range(2):
                        units[(g, h)] = pps.tile(
                            [128, 2, BLK], fp32, tag="mm", bufs=4, name=f"u{g}_{h}"
                        )
                # drains of the previous wave overlap this wave's matmuls
                for unit, u in pending:
                    drain(unit, u)
                pending.clear()
                for m in range(4):
                    ms = slice(m * 128, (m + 1) * 128)
                    for c in range(3):
                        for g in wave:
                            w, gi = g // 2, g % 2
                            lhsT = pcs[:, ms] if c == 2 else pabs[:, c, ms]
                            rhs = (
                                qcw[w][:, gi, :]
                                if c == 2
                                else qabw[w][:, gi, c, :]
                            )
                            nc.tensor.matmul(
                                units[(g, m // 2)][:, m % 2, :],
                                lhsT,
                                rhs,
                                start=(c == 0),
                                stop=(c == 2),
                            )
                for g in wave:
                    for h in range(2):
                        pending.append((units[(g, h)], g * 2 + h))

            # ship the first 8 acc columns while wave 2 drains
            nc.sync.dma_start(acc_d.ap()[:, 0:8], acc[:, 0:8])
            for unit, u in pending:
                drain(unit, u)
            nc.sync.dma_start(acc_d.ap()[:, 8:NUNIT], acc[:, 8:NUNIT])

    nc.compile()
    return nc


@functools.lru_cache(maxsize=1)
def _get_nc():
    return _build_bass()


def _pack_pq(y_true, y_pred, lookup):
    """Global [KT, B] P and Q fp32 matrices (see module docstring)."""
    yt = np.asarray(y_true).astype(np.int64)
    yp = np.asarray(y_pred).astype(np.float32)[:, :D_USE]
    lk = np.asarray(lookup).astype(np.float32)

    n = np.maximum(np.sqrt((yp * yp).sum(axis=1, keepdims=True)), NORM_EPS)
    zn = (yp / n).T  # [192, B]
    bg = (yt == -1).astype(np.float32)
    valid = (yt >= 0).astype(np.float32)
    idx = np.clip(yt, 0, L - 1)

    PG = np.zeros((KT, B), np.float32)
    QG = np.zeros((KT, B), np.float32)
    PG[0:D_USE] = zn
    QG[0:D_USE] = 0.5 * zn
    PG[192] = 1.0
    QG[192] = 0.5
    PG[193] = bg
    QG[193] = -0.01 - 0.18 * bg
    PG[194] = 1.0
    QG[194] = -0.01 * bg
    oh = np.zeros((L, B), np.float32)
    oh[idx, np.arange(B)] = valid
    PG[195 : 195 + L] = oh
    QG[195 : 195 + L] = -lk[:, idx] * valid[None, :]
    return PG, QG


def _host_inputs(y_true, y_pred, lookup):
    """Build the 8 per-core input maps."""
    PG, QG = _pack_pq(y_true, y_pred, lookup)

    in_maps = []
    for r in range(N_CORES):
        qcore = np.zeros((KT, NCOL), np.float32)
        for d in range(NSLOT):
            if d == 4 and r >= 4:
                continue  # padded slot stays zero
            cb = (r + d) % NB
            w = 1.0 if d == 0 else 2.0
            qcore[:, d * BLK : (d + 1) * BLK] = (
                w * QG[:, cb * BLK : (cb + 1) * BLK]
            )
        pcore = PG[:, r * BLK : (r + 1) * BLK]
        p8 = pcore.astype(FP8)
        q8 = qcore.astype(FP8)
        # device layout: i128 = [pab (p,c,m) | qab (p,g,c,n)]
        #                i67  = [pc  (p,m)   | qc  (p,g,n)]
        pab = p8[0:256].reshape(2, 128, BLK).transpose(1, 0, 2)
        qab = (
            q8[0:256]
            .reshape(2, 128, NSLOT, BLK)
            .transpose(1, 2, 0, 3)
        )
        i128 = np.concatenate(
            [pab.reshape(128, 2 * BLK), qab.reshape(128, NSLOT * 1024)], axis=1
        )
        i67 = np.concatenate([p8[256:KT], q8[256:KT]], axis=1)
        in_maps.append(
            {
                "i128": np.ascontiguousarray(i128),
                "i67": np.ascontiguousarray(i67),
            }
        )
    return in_maps


def _combine(outs, y_true, lookup):
    """outs: list of 8 dicts with 'acc' [128, NUNIT]."""
    yt = np.asarray(y_true).astype(np.int64)
    lk = np.asarray(lookup).astype(np.float64)

    total = 0.0
    for r in range(N_CORES):
        total += float(outs[r]["acc"].astype(np.float64).sum())

    # diagonal correction: latent_ii = 1, target_ii = 0.2 (bg) or lookup[y,y]
    bgm = yt == -1
    idx = np.clip(yt, 0, L - 1)
    tdiag = np.where(bgm, 0.2, lk[idx, idx])
    diag_sum = float(np.abs(1.0 - tdiag).sum())

    n_pairs = B * (B - 1) // 2
    return np.float32((total - diag_sum) / 2.0 / n_pairs)


def kernel(y_true, y_pred, lookup):
    from concourse.bass_utils import run_bass_kernel_spmd

    nc = _get_nc()
    in_maps = _host_inputs(y_true, y_pred, lookup)
    res = run_bass_kernel_spmd(nc, in_maps, core_ids=list(range(N_CORES)))
    return _combine(res.results, y_true, lookup)



# revision 3
# speedup vs baseline: 1.0401x; 1.0401x over previous
"""AffinityCosineLoss on 8 Trainium2 NeuronCores — fp8 DoubleRow matmul.

Math: with zn = l2norm(y_pred[:, :192]), latent = (zn@zn.T + 1)/2,
target[i,j] = 0.2 (both bg) / 0.01 (one bg) / lookup[y_i,y_j] (both valid),
loss = sum_{i<j} |latent - target| / (B*(B-1)/2).

The entire pairwise map latent - target is a single K=323 contraction
P.T @ Q, fully packed on the HOST (fp32 math, then fp8 cast):
  rows   0:192  P = zn_i.T            Q = 0.5 * zn_j.T
  row    192    P = 1                 Q = 0.5            (the +1/2 of latent)
  row    193    P = b_i               Q = -0.01 - 0.18*b_j
  row    194    P = 1                 Q = -0.01*b_j      (b = is_background)
  rows 195:323  P = onehot(y_i)       Q = -lookup[:, y_j] * valid_j
The asymmetric 1.0 x 0.5 const split keeps the fp8 constants exact.
K chunks: AB = rows 0:256 as ONE DoubleRow matmul (the fp8 interleave
virtualizes the PE array to 256 contraction rows), C = rows 256:323 (67)
as a plain matmul accumulated on top.

Sharding (triangle/cyclic): the 4096x4096 pair matrix is an 8x8 grid of
512x512 super-blocks. Core r computes blocks (r, (r+d) mod 8) for d=0..4;
the d=4 slot is zero-padded on cores 4..7. Off-diagonal slots count twice
(|M| is symmetric); the x2 weight is applied on the HOST per acc column.
Host: total = sum - diag_correction, /2, /npairs.

Device: 20 out-tiles [128,512], one PSUM bank each, rotating through all
8 banks. Drains (abs + sum into one acc column) alternate DVE
(tensor_reduce) / ACT (activation Abs accum_out) into separate per-engine
acc tiles so the two engines never write the same SBUF tile. Inputs ride
4 HWDGE DMAs (2 on sync, 2 on scalar) sized so the first-needed slot-0
data lands while zero-matmul PE warmup keeps the HAM clock ramping.
"""

import functools

import ml_dtypes
import numpy as np

B = 4096
D = 256
L = 128
D_USE = 192  # int(D * 0.75)
NB = 8  # super-block grid (512 rows each)
BLK = B // NB  # 512
NSLOT = 5  # col slots per core (d = 0..4)
N_CORES = 8
NORM_EPS = 1e-8

KT = 323  # contraction rows
KC = 67  # rows of chunk C
NWARM = 5  # PE warmup matmuls on zeros

USE_DR = True  # DoubleRow for the K=256 AB chunk

FP8 = ml_dtypes.float8_e4m3


def _build_bass():
    import concourse.bacc as bacc
    import concourse.mybir as mybir
    import concourse.tile as tile

    fp32 = mybir.dt.float32
    bf16 = mybir.dt.bfloat16
    f8 = mybir.dt.float8e4

    nc = bacc.Bacc("TRN2", debug=False, num_devices=N_CORES)

    # Split inputs so the slot-0 working set is its own early DMA.
    i128a_d = nc.dram_tensor("i128a", [128, 2048], f8, kind="ExternalInput")
    i128b_d = nc.dram_tensor("i128b", [128, 4096], f8, kind="ExternalInput")
    i67a_d = nc.dram_tensor("i67a", [KC, 1024], f8, kind="ExternalInput")
    i67b_d = nc.dram_tensor("i67b", [KC, 2048], f8, kind="ExternalInput")
    accv_d = nc.dram_tensor("accv", [128, 10], fp32, kind="ExternalOutput")
    acca_d = nc.dram_tensor("acca", [128, 10], fp32, kind="ExternalOutput")

    AX = mybir.AxisListType
    ALU = mybir.AluOpType
    ACTF = mybir.ActivationFunctionType
    DRM = mybir.MatmulPerfMode.DoubleRow

    with tile.TileContext(nc) as tc:
        with (
            tc.tile_pool(name="cst", bufs=1) as cst,
            tc.tile_pool(name="work", bufs=1) as work,
            tc.tile_pool(name="ps", bufs=1, space="PSUM") as pps,
        ):
            # [p, which(pab|qab g0), ko, n]
            t128a = work.tile([128, 2, 2, BLK], f8)
            # [p, g-1 (slots 1..4), ko, n]
            t128b = work.tile([128, 4, 2, BLK], f8)
            t67a = work.tile([KC, 2, BLK], f8)  # [p, which(pcs|qc g0), n]
            t67b = work.tile([KC, 4, BLK], f8)  # [p, g-1, n]
            accv = work.tile([128, 10], fp32)  # DVE-drained columns
            acca = work.tile([128, 10], fp32)  # ACT-drained columns

            # ---- input DMAs first: sync + scalar HWDGE rings in parallel
            nc.sync.dma_start(t128a[:], i128a_d.ap()[:])
            nc.scalar.dma_start(t67a[:], i67a_d.ap()[:])
            nc.sync.dma_start(t128b[:], i128b_d.ap()[:])
            nc.scalar.dma_start(t67b[:], i67b_d.ap()[:])

            # ---- engine warmup ----
            wz = cst.tile([128, BLK], f8)
            nc.vector.memset(wz[:], 0.0)
            wact = cst.tile([128, 1], fp32)
            nc.gpsimd.memset(wact[:], 1.0)
            wabs = cst.tile([128, 1], fp32)
            nc.scalar.activation(wabs[:], wact[:], ACTF.Abs)

            for wi in range(NWARM):
                wp = pps.tile([128, BLK], fp32, tag="mm", bufs=8, name=f"wp{wi}")
                nc.tensor.matmul(wp[:], wz[:, 0:128], wz[:], start=True, stop=True)

            # ---- main: per slot g, 4 DR matmuls then 4 C matmuls + drains
            for g in range(NSLOT):
                if g == 0:
                    qab = t128a[:, 1, :, :]
                    qc = t67a[:, 1, :]
                else:
                    qab = t128b[:, g - 1, :, :]
                    qc = t67b[:, g - 1, :]
                units = []
                for m in range(4):
                    ms = slice(m * 128, (m + 1) * 128)
                    u = pps.tile([128, BLK], fp32, tag="mm", bufs=8, name=f"u{g}_{m}")
                    if USE_DR:
                        nc.tensor.matmul(
                            u[:],
                            t128a[:, 0, :, ms],
                            qab,
                            start=True,
                            stop=False,
                            perf_mode=DRM,
                        )
                    else:
                        nc.tensor.matmul(
                            u[:], t128a[:, 0, 0, ms], qab[:, 0, :],
                            start=True, stop=False,
                        )
                        nc.tensor.matmul(
                            u[:], t128a[:, 0, 1, ms], qab[:, 1, :],
                            start=False, stop=False,
                        )
                    units.append(u)
                for m in range(4):
                    ms = slice(m * 128, (m + 1) * 128)
                    u = units[m]
                    nc.tensor.matmul(
                        u[:], t67a[:, 0, ms], qc, start=False, stop=True
                    )
                    uidx = g * 4 + m
                    col = uidx // 2
                    if uidx % 2 == 0:
                        nc.vector.tensor_reduce(
                            accv[:, col : col + 1],
                            u[:],
                            axis=AX.XY,
                            op=ALU.add,
                            apply_absolute_value=True,
                        )
                    else:
                        scr = work.tile([128, BLK], bf16, tag="scr", bufs=2)
                        nc.scalar.activation(
                            scr[:], u[:], ACTF.Abs, accum_out=acca[:, col : col + 1]
                        )

            nc.sync.dma_start(accv_d.ap()[:], accv[:])
            nc.scalar.dma_start(acca_d.ap()[:], acca[:])

    nc.compile()
    return nc


@functools.lru_cache(maxsize=1)
def _get_nc():
    return _build_bass()


def _pack_pq(y_true, y_pred, lookup):
    """Global [KT, B] P and Q fp32 matrices (see module docstring)."""
    yt = np.asarray(y_true).astype(np.int64)
    yp = np.asarray(y_pred).astype(np.float32)[:, :D_USE]
    lk = np.asarray(lookup).astype(np.float32)

    n = np.maximum(np.sqrt((yp * yp).sum(axis=1, keepdims=True)), NORM_EPS)
    zn = (yp / n).T  # [192, B]
    bg = (yt == -1).astype(np.float32)
    valid = (yt >= 0).astype(np.float32)
    idx = np.clip(yt, 0, L - 1)

    PG = np.zeros((KT, B), np.float32)
    QG = np.zeros((KT, B), np.float32)
    PG[0:D_USE] = zn
    QG[0:D_USE] = 0.5 * zn
    PG[192] = 1.0
    QG[192] = 0.5
    PG[193] = bg
    QG[193] = -0.01 - 0.18 * bg
    PG[194] = 1.0
    QG[194] = -0.01 * bg
    oh = np.zeros((L, B), np.float32)
    oh[idx, np.arange(B)] = valid
    PG[195 : 195 + L] = oh
    QG[195 : 195 + L] = -lk[:, idx] * valid[None, :]
    return PG, QG


def _fold_ko(a256):
    """[256, n] -> [128, 2, n] with row k at (k % 128, k // 128)."""
    n = a256.shape[1]
    return np.ascontiguousarray(a256.reshape(2, 128, n).transpose(1, 0, 2))


def _host_inputs(y_true, y_pred, lookup):
    """Build the 8 per-core input maps."""
    PG, QG = _pack_pq(y_true, y_pred, lookup)
    P8 = PG.astype(FP8)
    Q8 = QG.astype(FP8)

    in_maps = []
    for r in range(N_CORES):
        cols = [slice(((r + d) % NB) * BLK, ((r + d) % NB) * BLK + BLK)
                for d in range(NSLOT)]
        pab = _fold_ko(P8[0:256, r * BLK : (r + 1) * BLK])
        qabs = []
        qcs = []
        for d in range(NSLOT):
            if d == 4 and r >= 4:
                qabs.append(np.zeros((128, 2, BLK), FP8))
                qcs.append(np.zeros((KC, BLK), FP8))
            else:
                qabs.append(_fold_ko(Q8[0:256, cols[d]]))
                qcs.append(np.ascontiguousarray(Q8[256:KT, cols[d]]))
        pcs = np.ascontiguousarray(P8[256:KT, r * BLK : (r + 1) * BLK])

        i128a = np.concatenate(
            [pab.reshape(128, 1024), qabs[0].reshape(128, 1024)], axis=1
        )
        i128b = np.concatenate([q.reshape(128, 1024) for q in qabs[1:]], axis=1)
        i67a = np.concatenate([pcs, qcs[0]], axis=1)
        i67b = np.concatenate(qcs[1:], axis=1)
        in_maps.append(
            {
                "i128a": np.ascontiguousarray(i128a),
                "i128b": np.ascontiguousarray(i128b),
                "i67a": np.ascontiguousarray(i67a),
                "i67b": np.ascontiguousarray(i67b),
            }
        )
    return in_maps


# acc column weights: col c of accv holds unit 2c, of acca unit 2c+1;
# unit u covers slot g = u//4, weighted x2 except the diagonal slot g=0.
_WV = np.array([1.0 if (2 * c) // 4 == 0 else 2.0 for c in range(10)])
_WA = np.array([1.0 if (2 * c + 1) // 4 == 0 else 2.0 for c in range(10)])


def _combine(outs, y_true, lookup):
    """outs: list of 8 dicts with 'accv'/'acca' [128, 10]."""
    yt = np.asarray(y_true).astype(np.int64)
    lk = np.asarray(lookup).astype(np.float64)

    total = 0.0
    for r in range(N_CORES):
        av = outs[r]["accv"].astype(np.float64).sum(axis=0)
        aa = outs[r]["acca"].astype(np.float64).sum(axis=0)
        total += float((av * _WV).sum() + (aa * _WA).sum())

    # diagonal correction: latent_ii = 1, target_ii = 0.2 (bg) or lookup[y,y]
    bgm = yt == -1
    idx = np.clip(yt, 0, L - 1)
    tdiag = np.where(bgm, 0.2, lk[idx, idx])
    diag_sum = float(np.abs(1.0 - tdiag).sum())

    n_pairs = B * (B - 1) // 2
    return np.float32((total - diag_sum) / 2.0 / n_pairs)


def kernel(y_true, y_pred, lookup):
    from concourse.bass_utils import run_bass_kernel_spmd

    nc = _get_nc()
    in_maps = _host_inputs(y_true, y_pred, lookup)
    res = run_bass_kernel_spmd(nc, in_maps, core_ids=list(range(N_CORES)))
    return _combine(res.results, y_true, lookup)


# revision 7
# speedup vs baseline: 1.2548x; 1.2064x over previous
"""AffinityCosineLoss on 8 Trainium2 NeuronCores — fp8 DoubleRow matmul.

Math: with zn = l2norm(y_pred[:, :192]), latent = (zn@zn.T + 1)/2,
target[i,j] = 0.2 (both bg) / 0.01 (one bg) / lookup[y_i,y_j] (both valid),
loss = sum_{i<j} |latent - target| / (B*(B-1)/2).

The entire pairwise map latent - target is a single K=323 contraction
P.T @ Q, fully packed on the HOST (fp32 math, then fp8 cast):
  rows   0:192  P = zn_i.T            Q = 0.5 * zn_j.T
  row    192    P = 1                 Q = 0.5            (the +1/2 of latent)
  row    193    P = b_i               Q = -0.01 - 0.18*b_j
  row    194    P = 1                 Q = -0.01*b_j      (b = is_background)
  rows 195:323  P = onehot(y_i)       Q = -lookup[:, y_j] * valid_j
The asymmetric 1.0 x 0.5 const split keeps the fp8 constants exact.
K chunks: AB = rows 0:256 as ONE DoubleRow matmul (the fp8 interleave
virtualizes the PE array to 256 contraction rows), C = rows 256:323 (67)
as a plain matmul accumulated on top.

Sharding (triangle/cyclic): the 4096x4096 pair matrix is an 8x8 grid of
512x512 super-blocks. Core r computes blocks (r, (r+d) mod 8) for d=0..4;
the d=4 slot is zero-padded on cores 4..7. Off-diagonal slots count twice
(|M| is symmetric); the x2 weight is applied on the HOST per acc column.
Host: total = sum - diag_correction, /2, /npairs.

Device: 20 out-tiles [128,512], one PSUM bank each, rotating through all
8 banks. Drains (abs + sum into one acc column) alternate DVE
(tensor_reduce) / ACT (activation Abs accum_out) into separate per-engine
acc tiles so the two engines never write the same SBUF tile. Inputs ride
4 HWDGE DMAs (2 on sync, 2 on scalar) sized so the first-needed slot-0
data lands while zero-matmul PE warmup keeps the HAM clock ramping.
"""

import functools

import ml_dtypes
import numpy as np

B = 4096
D = 256
L = 128
D_USE = 192  # int(D * 0.75)
NB = 8  # super-block grid (512 rows each)
BLK = B // NB  # 512
NSLOT = 5  # col slots per core (d = 0..4)
N_CORES = 8
NORM_EPS = 1e-8

KT = 323  # contraction rows
KC = 67  # rows of chunk C
NWARM = 5  # PE warmup matmuls on zeros

USE_DR = True  # DoubleRow for the K=256 AB chunk

FP8 = ml_dtypes.float8_e4m3


def _build_bass():
    import concourse.bacc as bacc
    import concourse.mybir as mybir
    import concourse.tile as tile

    fp32 = mybir.dt.float32
    bf16 = mybir.dt.bfloat16
    f8 = mybir.dt.float8e4

    nc = bacc.Bacc("TRN2", debug=False, num_devices=N_CORES)

    # Split inputs so the slot-0 working set is its own early DMA. All
    # tensors are 128-partition: a [67, n] DMA lands on a single SDMA
    # engine (~22 GB/s); zero-padding chunk C to 128 rows fans it across
    # all 16 engines.
    ipab_d = nc.dram_tensor("ipab", [128, 1024], f8, kind="ExternalInput")
    iqab0_d = nc.dram_tensor("iqab0", [128, 1024], f8, kind="ExternalInput")
    i128b_d = nc.dram_tensor("i128b", [128, 4096], f8, kind="ExternalInput")
    i67a_d = nc.dram_tensor("i67a", [128, 1024], f8, kind="ExternalInput")
    i67b_d = nc.dram_tensor("i67b", [128, 2048], f8, kind="ExternalInput")
    accv_d = nc.dram_tensor("accv", [128, 10], fp32, kind="ExternalOutput")
    acca_d = nc.dram_tensor("acca", [128, 10], fp32, kind="ExternalOutput")

    AX = mybir.AxisListType
    ALU = mybir.AluOpType
    ACTF = mybir.ActivationFunctionType
    DRM = mybir.MatmulPerfMode.DoubleRow

    with tile.TileContext(nc) as tc:
        with (
            tc.tile_pool(name="cst", bufs=1) as cst,
            tc.tile_pool(name="work", bufs=1) as work,
            tc.tile_pool(name="ps", bufs=1, space="PSUM") as pps,
        ):
            tpab = work.tile([128, 2, BLK], f8)  # [p, ko, m]
            tqab0 = work.tile([128, 2, BLK], f8)  # [p, ko, n]
            # [p, g-1 (slots 1..4), ko, n]
            t128b = work.tile([128, 4, 2, BLK], f8)
            t67a = work.tile([128, 2, BLK], f8)  # [p, which(pcs|qc g0), n]
            t67b = work.tile([128, 4, BLK], f8)  # [p, g-1, n]
            accv = work.tile([128, 10], fp32)  # DVE-drained columns
            acca = work.tile([128, 10], fp32)  # ACT-drained columns

            # ---- input DMAs first: sync + scalar HWDGE rings in parallel
            nc.sync.dma_start(tpab[:], ipab_d.ap()[:])
            nc.scalar.dma_start(t67a[:], i67a_d.ap()[:])
            nc.sync.dma_start(tqab0[:], iqab0_d.ap()[:])
            nc.sync.dma_start(t128b[:], i128b_d.ap()[:])
            nc.scalar.dma_start(t67b[:], i67b_d.ap()[:])

            # ---- engine warmup ----
            wz = cst.tile([128, BLK], f8)
            nc.vector.memset(wz[:], 0.0)
            wact = cst.tile([128, 1], fp32)
            nc.gpsimd.memset(wact[:], 1.0)
            wabs = cst.tile([128, 1], fp32)
            nc.scalar.activation(wabs[:], wact[:], ACTF.Abs)

            for wi in range(NWARM):
                wp = pps.tile([128, BLK], fp32, tag="mm", bufs=8, name=f"wp{wi}")
                nc.tensor.matmul(wp[:], wz[:, 0:128], wz[:], start=True, stop=True)

            # ---- main: per slot g, 4 DR matmuls then 4 C matmuls + drains
            for g in range(NSLOT):
                if g == 0:
                    qab = tqab0[:, :, :]
                    qc = t67a[:, 1, :]
                else:
                    qab = t128b[:, g - 1, :, :]
                    qc = t67b[:, g - 1, :]
                units = []
                for m in range(4):
                    ms = slice(m * 128, (m + 1) * 128)
                    u = pps.tile([128, BLK], fp32, tag="mm", bufs=8, name=f"u{g}_{m}")
                    if USE_DR:
                        nc.tensor.matmul(
                            u[:],
                            tpab[:, :, ms],
                            qab,
                            start=True,
                            stop=False,
                            perf_mode=DRM,
                        )
                    else:
                        nc.tensor.matmul(
                            u[:], tpab[:, 0, ms], qab[:, 0, :],
                            start=True, stop=False,
                        )
                        nc.tensor.matmul(
                            u[:], tpab[:, 1, ms], qab[:, 1, :],
                            start=False, stop=False,
                        )
                    units.append(u)
                for m in range(4):
                    ms = slice(m * 128, (m + 1) * 128)
                    u = units[m]
                    nc.tensor.matmul(
                        u[:], t67a[:, 0, ms], qc, start=False, stop=True
                    )
                    uidx = g * 4 + m
                    col = uidx // 2
                    if uidx % 2 == 0:
                        nc.vector.tensor_reduce(
                            accv[:, col : col + 1],
                            u[:],
                            axis=AX.XY,
                            op=ALU.add,
                            apply_absolute_value=True,
                        )
                    else:
                        scr = work.tile([128, BLK], bf16, tag="scr", bufs=2)
                        nc.scalar.activation(
                            scr[:], u[:], ACTF.Abs, accum_out=acca[:, col : col + 1]
                        )

            nc.sync.dma_start(accv_d.ap()[:], accv[:])
            nc.scalar.dma_start(acca_d.ap()[:], acca[:])

    nc.compile()
    return nc


@functools.lru_cache(maxsize=1)
def _get_nc():
    return _build_bass()


def _pack_pq(y_true, y_pred, lookup):
    """Global [KT, B] P and Q fp32 matrices (see module docstring)."""
    yt = np.asarray(y_true).astype(np.int64)
    yp = np.asarray(y_pred).astype(np.float32)[:, :D_USE]
    lk = np.asarray(lookup).astype(np.float32)

    n = np.maximum(np.sqrt((yp * yp).sum(axis=1, keepdims=True)), NORM_EPS)
    zn = (yp / n).T  # [192, B]
    bg = (yt == -1).astype(np.float32)
    valid = (yt >= 0).astype(np.float32)
    idx = np.clip(yt, 0, L - 1)

    PG = np.zeros((KT, B), np.float32)
    QG = np.zeros((KT, B), np.float32)
    PG[0:D_USE] = zn
    QG[0:D_USE] = 0.5 * zn
    PG[192] = 1.0
    QG[192] = 0.5
    PG[193] = bg
    QG[193] = -0.01 - 0.18 * bg
    PG[194] = 1.0
    QG[194] = -0.01 * bg
    oh = np.zeros((L, B), np.float32)
    oh[idx, np.arange(B)] = valid
    PG[195 : 195 + L] = oh
    QG[195 : 195 + L] = -lk[:, idx] * valid[None, :]
    return PG, QG


def _fold_ko(a256):
    """[256, n] -> [128, 2, n] with row k at (k % 128, k // 128)."""
    n = a256.shape[1]
    return np.ascontiguousarray(a256.reshape(2, 128, n).transpose(1, 0, 2))


def _host_inputs(y_true, y_pred, lookup):
    """Build the 8 per-core input maps."""
    PG, QG = _pack_pq(y_true, y_pred, lookup)
    P8 = PG.astype(FP8)
    Q8 = QG.astype(FP8)

    # chunk C zero-padded to 128 rows so its DMAs fan across all engines
    PC = np.zeros((128, B), FP8)
    QC = np.zeros((128, B), FP8)
    PC[0:KC] = P8[256:KT]
    QC[0:KC] = Q8[256:KT]

    in_maps = []
    for r in range(N_CORES):
        cols = [slice(((r + d) % NB) * BLK, ((r + d) % NB) * BLK + BLK)
                for d in range(NSLOT)]
        pab = _fold_ko(P8[0:256, r * BLK : (r + 1) * BLK])
        qabs = []
        qcs = []
        for d in range(NSLOT):
            if d == 4 and r >= 4:
                qabs.append(np.zeros((128, 2, BLK), FP8))
                qcs.append(np.zeros((128, BLK), FP8))
            else:
                qabs.append(_fold_ko(Q8[0:256, cols[d]]))
                qcs.append(np.ascontiguousarray(QC[:, cols[d]]))
        pcs = np.ascontiguousarray(PC[:, r * BLK : (r + 1) * BLK])

        i128b = np.concatenate([q.reshape(128, 1024) for q in qabs[1:]], axis=1)
        i67a = np.concatenate([pcs, qcs[0]], axis=1)
        i67b = np.concatenate(qcs[1:], axis=1)
        in_maps.append(
            {
                "ipab": np.ascontiguousarray(pab.reshape(128, 1024)),
                "iqab0": np.ascontiguousarray(qabs[0].reshape(128, 1024)),
                "i128b": np.ascontiguousarray(i128b),
                "i67a": np.ascontiguousarray(i67a),
                "i67b": np.ascontiguousarray(i67b),
            }
        )
    return in_maps


# acc column weights: col c of accv holds unit 2c, of acca unit 2c+1;
# unit u covers slot g = u//4, weighted x2 except the diagonal slot g=0.
_WV = np.array([1.0 if (2 * c) // 4 == 0 else 2.0 for c in range(10)])
_WA = np.array([1.0 if (2 * c + 1) // 4 == 0 else 2.0 for c in range(10)])


def _combine(outs, y_true, lookup):
    """outs: list of 8 dicts with 'accv'/'acca' [128, 10]."""
    yt = np.asarray(y_true).astype(np.int64)
    lk = np.asarray(lookup).astype(np.float64)

    total = 0.0
    for r in range(N_CORES):
        av = outs[r]["accv"].astype(np.float64).sum(axis=0)
        aa = outs[r]["acca"].astype(np.float64).sum(axis=0)
        total += float((av * _WV).sum() + (aa * _WA).sum())

    # diagonal correction: latent_ii = 1, target_ii = 0.2 (bg) or lookup[y,y]
    bgm = yt == -1
    idx = np.clip(yt, 0, L - 1)
    tdiag = np.where(bgm, 0.2, lk[idx, idx])
    diag_sum = float(np.abs(1.0 - tdiag).sum())

    n_pairs = B * (B - 1) // 2
    return np.float32((total - diag_sum) / 2.0 / n_pairs)


def kernel(y_true, y_pred, lookup):
    from concourse.bass_utils import run_bass_kernel_spmd

    nc = _get_nc()
    in_maps = _host_inputs(y_true, y_pred, lookup)
    res = run_bass_kernel_spmd(nc, in_maps, core_ids=list(range(N_CORES)))
    return _combine(res.results, y_true, lookup)
